# revision 18
# baseline (speedup 1.0000x reference)
"""Trainium2 Bass kernel for nn_Attention_11046655885519.

Self-contained: takes full (unsharded) inputs, shards across 8 NeuronCores
(batch x query-half), runs one SPMD NEFF, reassembles the full output.

Design:
- BN folded into conv weights/biases on host (numpy).
- Scores computed transposed (keys on partitions): softmax needs no
  max-subtraction and no transposes; exp runs on ScalarE straight from PSUM
  (ScalarE is the bottleneck engine: ~83k columns of exp per core).
- Heads in pairs: K=8 score matmuls row-tiled via tile_position; one exp
  ACTIVATE per (pair, key-tile, query-chunk) spanning both heads' PSUM
  banks; query chunks {512, 288} keep ACTIVATE rows long.
- PV matmuls (M=32: v^T | ones | zeros) accumulate per-head PSUM banks;
  the ones column yields the softmax denominator free; stream_shuffle
  broadcasts it, reciprocal+multiply divide in place.
- Attention output stays blocked; proj 1x1 absorbs the permutation via
  host-built weights. Depthwise 3x3 = 9 diagonal-matrix PE matmuls.
- float32r matmuls (full PE rate); fp32 fallback via KERNEL_MM_RAW=0.
- Software-pipelined: S^T emitted one step ahead (PE is in-order), exp
  table preloaded, phase 0 interleaved with the pipeline start.
"""

import os
import sys

sys.path.insert(0, "/opt/trn_rl_repo")
os.environ.setdefault("MYCRO_LOCAL_CACHE", "1")

import numpy as np

DIM = 128
HEADS = 8
HD = 16
KD = 8
EPS = 1e-3
SCALE = KD ** -0.5
N = 1600  # 40*40
Q = 800  # queries per core
NCORES = 8
GROUPS = [(0, 1), (2, 3), (4, 5), (6, 7)]
QCS = [(0, 400), (400, 400)]  # query chunks (start, width)
TAPS = [(dy, dx) for dy in (-1, 0, 1) for dx in (-1, 0, 1)]
NKT = 13  # key tiles: 12 x 128 + 64

_CACHE = {}


def _build_nc(mm_raw=True, reps=1):
    """Build the Bass module (one NEFF, SPMD across 8 cores)."""
    from contextlib import ExitStack

    import concourse.bass as bass
    import concourse.tile as tile
    from concourse import bacc, mybir

    f32 = mybir.dt.float32
    f32r = mybir.dt.float32r
    Exp = mybir.ActivationFunctionType.Exp
    add_op = mybir.AluOpType.add

    RD = f32r if mm_raw else f32

    nc = bacc.Bacc(
        "TRN2",
        target_bir_lowering=False,
        debug=False,
        enable_asserts=False,
        num_devices=NCORES,
    )

    def din(name, shape, dt=None):
        return nc.dram_tensor(name, shape, dt or f32, kind="ExternalInput").ap()

    x_ext_d = din("x_ext", [128, N + 80], RD)
    wk_a_d = din("wk_a", [128, 128], RD)
    wk_b_d = din("wk_b", [128, 128], RD)
    wq_a_d = din("wq_a", [128, 128], RD)
    wq_b_d = din("wq_b", [128, 128], RD)
    wv_d = din("wv", [128, 128], RD)
    wpg_d = din("wpg", [32, 8 * 128], RD)  # blocked proj weights per (g, j)
    wp_d = din("wp", [128, 128], RD)  # dense proj weights (pe path)
    ident_d = din("ident", [128, 128], RD)
    dw_d = din("dw", [128, 9 * 128], RD)
    bk_a_d = din("bk_a", [128, 1])
    bk_b_d = din("bk_b", [128, 1])
    bq_a_d = din("bq_a", [128, 1])
    bq_b_d = din("bq_b", [128, 1])
    bv_d = din("bv", [128, 1])
    by_d = din("by", [128, 1])
    mask_d = din("mask", [128, 80])
    y_d = nc.dram_tensor("y", [128, Q], f32, kind="ExternalOutput").ap()

    with ExitStack() as ctx:
        tc = ctx.enter_context(tile.TileContext(nc))
        consts = ctx.enter_context(tc.tile_pool(name="consts", bufs=1))
        work = ctx.enter_context(tc.tile_pool(name="work", bufs=1))

        def load(dram_ap, shape, nm, dt=None):
            t = consts.tile(shape, dt or f32, name=nm)
            nc.sync.dma_start(out=t, in_=dram_ap)
            return t

        x_sb = load(x_ext_d, [128, N + 80], "x_sb", RD)
        wk_a = load(wk_a_d, [128, 128], "wk_a_sb", RD)
        wk_b = load(wk_b_d, [128, 128], "wk_b_sb", RD)
        wq_a = load(wq_a_d, [128, 128], "wq_a_sb", RD)
        wq_b = load(wq_b_d, [128, 128], "wq_b_sb", RD)
        wv = load(wv_d, [128, 128], "wv_sb", RD)
        wpg = load(wpg_d, [32, 8, 128], "wpg_sb", RD)
        wp = load(wp_d, [128, 128], "wp_sb", RD)
        ident = load(ident_d, [128, 128], "ident_sb", RD)
        dw = load(dw_d, [128, 9 * 128], "dw_sb", RD)
        bk_a = load(bk_a_d, [128, 1], "bk_a_sb")
        bk_b = load(bk_b_d, [128, 1], "bk_b_sb")
        bq_a = load(bq_a_d, [128, 1], "bq_a_sb")
        bq_b = load(bq_b_d, [128, 1], "bq_b_sb")
        bv = load(bv_d, [128, 1], "bv_sb")
        by = load(by_d, [128, 1], "by_sb")
        mask = load(mask_d, [128, 80], "mask_sb")

        ka = work.tile([128, N], RD, name="ka")
        kb = work.tile([128, N], RD, name="kb")
        qa = work.tile([128, Q], RD, name="qa")
        qb = work.tile([128, Q], RD, name="qb")
        v_sb = work.tile([128, N + 80], RD, name="v_sb")
        vt = work.tile([128, NKT, 8, 32], RD, name="vt")
        vpad = work.tile([128, 22, 42], RD, name="vpad")
        pe_sb = work.tile([128, Q], RD, name="pe_sb")
        div_sb = work.tile([32, 2, 4, 2, 512], RD, name="div_sb")
        y_sb = work.tile([128, Q], f32, name="y_sb")
        warm = work.tile([128, 1], f32, name="warm")

        # pools: st slots 2 banks x2, acc 2 banks x2 -> exactly 8 PSUM banks
        stp = ctx.enter_context(tc.tile_pool(name="stp", bufs=2, space="PSUM"))
        accp = ctx.enter_context(tc.tile_pool(name="accp", bufs=2, space="PSUM"))
        expp = ctx.enter_context(tc.tile_pool(name="expp", bufs=4))
        divp = ctx.enter_context(tc.tile_pool(name="divp", bufs=2))

        def body():
            # preload the exp table set while phase 0 runs
            nc.scalar.activation(out=warm, in_=bk_a[:, 0:1], func=Exp)

            steps = [
                (qc, g, kt)
                for qc in range(2)
                for g in range(len(GROUPS))
                for kt in range(NKT)
            ]
            sts = {}

            def emit_st(i):
                qc, g, kt = steps[i]
                kn = 128 if kt < 12 else 64
                ks = slice(kt * 128, kt * 128 + kn)
                q0, qw = QCS[qc]
                st = stp.tile([128, 2, 512], f32, name="st", tag="st")
                for j, h in enumerate(GROUPS[g]):
                    ktile = ka if h < 4 else kb
                    qtile = qa if h < 4 else qb
                    base = 32 * (h % 4)
                    nc.tensor.matmul(
                        st[:kn, j, 0:qw],
                        ktile[base : base + 8, ks],
                        qtile[base : base + 8, q0 : q0 + qw],
                        start=True, stop=True,
                        tile_position=(base, 0),
                    )
                sts[i] = st

            # ---- phase 0 interleaved with pipeline start ----
            def conv(dst, w, b, rhs, total, chunk):
                for c0 in range(0, total, chunk):
                    cw = min(chunk, total - c0)
                    ps = stp.tile([128, 512], f32, name="convps", tag="st")
                    nc.tensor.matmul(
                        ps[:, :cw], w[:, :], rhs[:, c0 : c0 + cw],
                        start=True, stop=True,
                    )
                    nc.vector.tensor_scalar(
                        out=dst[:, c0 : c0 + cw], in0=ps[:, :cw],
                        scalar1=b[:, 0:1], scalar2=None, op0=add_op,
                    )

            conv(ka, wk_a, bk_a, x_sb[:, :N], N, 400)
            conv(qa, wq_a, bq_a, x_sb[:, :Q], Q, 400)
            conv(kb, wk_b, bk_b, x_sb[:, :N], N, 400)
            conv(qb, wq_b, bq_b, x_sb[:, :Q], Q, 400)
            emit_st(0)
            emit_st(1)
            conv(v_sb, wv, bv, x_sb[:, : N + 80], N + 80, 420)

            # V^T tiles: per-head 32-wide blocks (data 0:16 | ones@16 | zeros)
            nc.vector.memset(vt.bitcast(f32), 0.0)
            nc.vector.memset(vt[:, :, :, 16:17].bitcast(f32), 1.0)
            for kt in range(NKT):
                kn = 128 if kt < 12 else 64
                tp = accp.tile([128, 128], RD, name="trps", tag="acc")
                nc.tensor.transpose(
                    tp[:kn, :], v_sb[:, kt * 128 : kt * 128 + kn], ident[:, :]
                )
                nc.vector.tensor_copy(
                    out=vt[:kn, kt, :, 0:16],
                    in_=tp[:kn, :].rearrange("p (h d) -> p h d", h=8),
                )
            emit_st(2)
            emit_st(3)

            # vpad: zero-padded 22x42 image of own rows + halo
            nc.vector.memset(vpad.bitcast(f32), 0.0)
            nc.vector.tensor_copy(
                out=vpad[:, 1:21, 1:41],
                in_=v_sb[:, 0:Q].rearrange("p (r c) -> p r c", r=20),
            )
            nc.vector.tensor_mul(vpad[:, 0, 1:41], v_sb[:, N : N + 40], mask[:, 0:40])
            nc.vector.tensor_mul(
                vpad[:, 21, 1:41], v_sb[:, N + 40 : N + 80], mask[:, 40:80]
            )

            # depthwise 3x3 as 9 diagonal matmuls, accumulated in PSUM
            for r0, nr in ((0, 12), (12, 8)):
                dps = stp.tile([128, 512], f32, name="dwps", tag="st")
                nn = nr * 40
                for t, (dy, dx) in enumerate(TAPS):
                    nc.tensor.matmul(
                        dps[:, :nn],
                        dw[:, t * 128 : (t + 1) * 128],
                        vpad[:, 1 + r0 + dy : 1 + r0 + dy + nr, 1 + dx : 41 + dx],
                        start=(t == 0), stop=(t == 8),
                        skip_group_check=True,
                    )
                nc.vector.tensor_copy(
                    out=pe_sb[:, r0 * 40 : r0 * 40 + nn], in_=dps[:, :nn]
                )

            # ---- phase 1: software-pipelined attention ----
            bcast16 = [16] * 32
            EMITTED = 4
            LOOKAHEAD = 1
            accs = {}
            for i, (qc, g, kt) in enumerate(steps):
                kn = 128 if kt < 12 else 64
                q0, qw = QCS[qc]
                if kt == 0:
                    accs[(qc, g)] = accp.tile(
                        [128, 2, 512], f32, name="acc", tag="acc"
                    )
                acc = accs[(qc, g)]
                nxt = i + LOOKAHEAD + 1
                if nxt >= EMITTED and nxt < len(steps):
                    emit_st(nxt)
                st = sts.pop(i)
                ex = expp.tile([128, 2, 400], RD, name="ex", tag="ex")
                nc.scalar.activation(
                    out=ex[:kn, :, 0:qw], in_=st[:kn, :, 0:qw], func=Exp
                )
                for j, h in enumerate(GROUPS[g]):
                    nc.tensor.matmul(
                        acc[0:32, j, 0:qw],
                        vt[:kn, kt, h, :],
                        ex[:kn, j, 0:qw],
                        start=(kt == 0), stop=(kt == 12),
                        skip_group_check=True,
                    )
                if kt < 12:
                    continue
                # group epilogue: denominator broadcast per 32-block,
                # reciprocal, divide -> blocked attention output in SBUF
                acc = accs.pop((qc, g))
                bden = divp.tile([32, 2, 512], f32, name="bden", tag="bden")
                nc.vector.stream_shuffle(
                    bden[:, :, 0:qw], acc[0:32, :, 0:qw], bcast16
                )
                rbc = divp.tile([32, 2, 512], f32, name="rbc", tag="rbc")
                scr = divp.tile([32, 2, 512], f32, name="scr", tag="scr")
                nc.vector.reciprocal_approx_accurate(
                    out=rbc[:, :, 0:qw], in_=bden[:, :, 0:qw],
                    scratch=scr[:, :, 0:qw],
                )
                nc.vector.tensor_mul(
                    div_sb[:, qc, g, :, 0:qw], acc[0:32, :, 0:qw], rbc[:, :, 0:qw]
                )
                if g != len(GROUPS) - 1:
                    continue
                # qc epilogue: proj = 8 blocked + 1 dense (pe) matmuls
                pj = stp.tile([128, 512], f32, name="pjps", tag="st")
                for pg in range(4):
                    for pjj in range(2):
                        nc.tensor.matmul(
                            pj[:, 0:qw],
                            wpg[:, 2 * pg + pjj, :],
                            div_sb[:, qc, pg, pjj, 0:qw],
                            start=(pg == 0 and pjj == 0), stop=False,
                            skip_group_check=True,
                        )
                nc.tensor.matmul(
                    pj[:, 0:qw], wp[:, :], pe_sb[:, q0 : q0 + qw],
                    start=False, stop=True,
                    skip_group_check=True,
                )
                nc.vector.tensor_scalar(
                    out=y_sb[:, q0 : q0 + qw], in0=pj[:, 0:qw],
                    scalar1=by[:, 0:1], scalar2=None, op0=add_op,
                )
                nc.sync.dma_start(
                    out=y_d[:, q0 : q0 + qw], in_=y_sb[:, q0 : q0 + qw]
                )

        if reps > 1:
            with tc.For_i(0, reps, 1):
                body()
        else:
            body()

    nc.compile()
    return nc


def _prep_inputs(inputs):
    """Fold BN into weights/biases, permute/shard per core. Pure numpy."""
    f = np.float32
    x = np.asarray(inputs["x"], f).reshape(4, DIM, N)

    def bn_fold(g, b, m, v):
        s = np.asarray(g, f) / np.sqrt(np.asarray(v, f) + EPS)
        return s, np.asarray(b, f) - s * np.asarray(m, f)

    s_qkv, beta_qkv = bn_fold(
        inputs["qkv_g"], inputs["qkv_b"], inputs["qkv_m"], inputs["qkv_v"]
    )
    W = np.asarray(inputs["qkv_w"], f)[:, :, 0, 0] * s_qkv[:, None]  # [256,128]

    q_rows = np.array([32 * h + j for h in range(8) for j in range(8)])
    k_rows = np.array([32 * h + 8 + j for h in range(8) for j in range(8)])
    v_rows = np.array([32 * h + 16 + j for h in range(8) for j in range(16)])

    Wq = W[q_rows] * SCALE
    bq = beta_qkv[q_rows] * SCALE
    Wk = W[k_rows]
    bk = beta_qkv[k_rows]
    Wv = W[v_rows]  # vch order
    bv = beta_qkv[v_rows]

    def arrange(Wm, bm, h0):
        # heads h0..h0+4 at partition bases 0,32,64,96 (8 rows each)
        wt = np.zeros((128, 128), f)
        bt = np.zeros((128, 1), f)
        for i in range(4):
            h = h0 + i
            wt[:, 32 * i : 32 * i + 8] = Wm[8 * h : 8 * h + 8].T
            bt[32 * i : 32 * i + 8, 0] = bm[8 * h : 8 * h + 8]
        return wt, bt

    wk_a, bk_a = arrange(Wk, bk, 0)
    wk_b, bk_b = arrange(Wk, bk, 4)
    wq_a, bq_a = arrange(Wq, bq, 0)
    wq_b, bq_b = arrange(Wq, bq, 4)

    wv = np.ascontiguousarray(Wv.T)  # lhsT [in, vch]
    bv_c = bv.reshape(128, 1)

    s_pe, beta_pe = bn_fold(
        inputs["pe_g"], inputs["pe_b"], inputs["pe_m"], inputs["pe_v"]
    )
    dwW = np.asarray(inputs["pe_w"], f)[:, 0, :, :] * s_pe[:, None, None]
    dw = np.zeros((128, 9 * 128), f)
    for t, (dy, dx) in enumerate(TAPS):
        dw[np.arange(128), t * 128 + np.arange(128)] = dwW[:, dy + 1, dx + 1]

    s_pr, beta_pr = bn_fold(
        inputs["proj_g"], inputs["proj_b"], inputs["proj_m"], inputs["proj_v"]
    )
    Wp = np.asarray(inputs["proj_w"], f)[:, :, 0, 0] * s_pr[:, None]
    by = (beta_pr + Wp @ beta_pe).reshape(128, 1)
    wp = np.ascontiguousarray(Wp.T)

    # blocked proj weights: (g, j) block rows 0:16 -> vch of h(g,j)
    wpg = np.zeros((32, 8 * 128), f)
    for g, heads in enumerate(GROUPS):
        for j, h in enumerate(heads):
            for d in range(16):
                wpg[d, 128 * (2 * g + j) : 128 * (2 * g + j + 1)] = Wp[:, 16 * h + d]

    ident = np.eye(128, dtype=f)

    shared = dict(
        wk_a=wk_a, wk_b=wk_b, wq_a=wq_a, wq_b=wq_b, wv=wv, wp=wp, wpg=wpg,
        ident=ident, dw=dw, bk_a=bk_a, bk_b=bk_b, bq_a=bq_a, bq_b=bq_b,
        bv=bv_c, by=by,
    )

    in_maps = []
    for c in range(NCORES):
        b, half = c // 2, c % 2
        xb = x[b]
        if half == 0:
            xp = xb
            pre = np.zeros((128, 40), f)
            post = xb[:, 800:840]
            msk = np.concatenate(
                [np.zeros((128, 40), f), np.ones((128, 40), f)], axis=1
            )
        else:
            xp = np.concatenate([xb[:, 800:1600], xb[:, 0:800]], axis=1)
            pre = xb[:, 760:800]
            post = np.zeros((128, 40), f)
            msk = np.concatenate(
                [np.ones((128, 40), f), np.zeros((128, 40), f)], axis=1
            )
        x_ext = np.ascontiguousarray(np.concatenate([xp, pre, post], axis=1))
        m = dict(shared)
        m["x_ext"] = x_ext
        m["mask"] = msk
        in_maps.append(m)
    return in_maps


def _get_nc():
    mm_raw = os.environ.get("KERNEL_MM_RAW", "1") == "1"
    reps = int(os.environ.get("KERNEL_BENCH_REPS", "1"))
    key = ("nc", mm_raw, reps)
    if key not in _CACHE:
        _CACHE[key] = _build_nc(mm_raw=mm_raw, reps=reps)
    return _CACHE[key]


def _run(inputs, trace=False):
    from concourse import bass_utils

    nc = _get_nc()
    in_maps = _prep_inputs(inputs)
    res = bass_utils.run_bass_kernel_spmd(
        nc, in_maps, core_ids=list(range(NCORES)), trace=trace
    )
    y = np.zeros((4, DIM, N), np.float32)
    for c in range(NCORES):
        b, half = c // 2, c % 2
        y[b, :, half * 800 : half * 800 + 800] = res.results[c]["y"]
    return y.reshape(4, DIM, 40, 40), res


def kernel(**inputs) -> np.ndarray:
    y, _ = _run(inputs, trace=False)
    return y
